# revision 4
# baseline (speedup 1.0000x reference)
"""BERT self-attention on 8 Trainium2 NeuronCores.

Sharding: data-parallel over batch (B=8 -> 1 batch element per core).
Every core runs the same single-core Bass kernel on its own batch slice;
weights/mask are replicated. The final output is a host-side stack.

Per-core algorithm (S=1024, HID=1024, NH=16, HD=64), all matmuls bf16
with fp32 PSUM accumulation:

  xT = X^T (host-transposed, bf16)             [HID, S]
  Q^T = Wq^T @ X^T   (lhsT = Wq col-chunks)    [HID, S]  (+bq per-partition)
  K^T = Wk^T @ X^T                             [HID, S]  (+bk per-partition)
  V   = X @ Wv       (lhsT = xT)               [S, HID]  (+bv broadcast)
  per head pair (2c, 2c+1) living in hid chunk c (head 2c in partitions
  0:64, head 2c+1 in 64:128 of qT/kT chunk c):
    S^T = K_h @ Q_h^T  -- per (head, kt, q-half): TWO col-tiled matmuls
          with 64-row contraction and 64-col stationaries at PE
          tile_position (r, 0) and (r, 64); they share the moving Q
          stream and run CONCURRENTLY on the PE array, so a head's
          512-q-col score block costs ~226ns instead of ~452ns.
    P^T = exp(S^T/8 + mask[k])   (ScalarE; the bottleneck: 128 exps x
          ~1.1us = 142us of ACT time; the schedule is built to start
          this stream early and keep it dense)
    ctx = P^T.T @ [V_h | 1]  (ones column yields the softmax denominator)
    out[:, h] = ctx[:, :64] * (1/Z)

Pipeline: wq/wk are DMA'd as column chunks so Q(0)/K(0) start right
after xT lands (~14us); warmup matmuls keep the PE clock ramped through
the fill; V projections and QK(c+1) are emitted in small per-kt slots
inside each chunk's score/exp loop so no blob ever blocks the in-order
PE queue ahead of the exp stream; ctx for chunk c runs under chunk
c+2's exp window (c7 carries ctx(5) and ctx(6); only ctx(7) trails).
"""

import functools

import numpy as np
import ml_dtypes

B, S, HID = 8, 1024, 1024
NH, HD = 16, 64
P = 128
NCH = HID // P  # hid chunks (8)
NKT = S // P  # key tiles (8)
NQT = S // P  # query tiles (8)
VROW = NH * (HD + 1)  # 1040: per-seq-chunk V row: 16 x (64 V cols + ones col)
N_CORES = 8

SCALE = 1.0 / float(np.sqrt(HD))


@functools.lru_cache(maxsize=None)
def _build(has_bv: bool):
    import concourse.bass as bass
    import concourse.tile as tile
    from concourse import bacc, mybir
    from contextlib import ExitStack

    fp32 = mybir.dt.float32
    bf16 = mybir.dt.bfloat16
    EXP = mybir.ActivationFunctionType.Exp

    nc = bacc.Bacc("TRN2", target_bir_lowering=False)

    xT = nc.dram_tensor("xT", [HID, S], bf16, kind="ExternalInput")
    # wq/wk host-pre-shuffled to [c_out, p(hid_in%128), kc(hid_in//128), col]
    # so one contiguous DMA delivers the full column chunk c (everything
    # Q(c)/K(c) need), letting chunk 0's projections start right after xT.
    wq = nc.dram_tensor("wq", [NCH, P, NCH, P], bf16, kind="ExternalInput")
    wk = nc.dram_tensor("wk", [NCH, P, NCH, P], bf16, kind="ExternalInput")
    wv = nc.dram_tensor("wv", [HID, HID], bf16, kind="ExternalInput")
    bq = nc.dram_tensor("bq", [P, NCH], fp32, kind="ExternalInput")
    bk = nc.dram_tensor("bk", [P, NCH], fp32, kind="ExternalInput")
    bv = nc.dram_tensor("bv", [HID], fp32, kind="ExternalInput") if has_bv else None
    mask = nc.dram_tensor("mask", [P, NKT], fp32, kind="ExternalInput")
    out = nc.dram_tensor("out", [S, HID], fp32, kind="ExternalOutput")

    with tile.TileContext(nc) as tc, ExitStack() as ctx:
        persist = ctx.enter_context(tc.tile_pool(name="persist", bufs=1))
        misc = ctx.enter_context(tc.tile_pool(name="misc", bufs=8))
        qT_pool = ctx.enter_context(tc.tile_pool(name="qT", bufs=2))
        kT_pool = ctx.enter_context(tc.tile_pool(name="kT", bufs=2))
        pT_pool = ctx.enter_context(tc.tile_pool(name="pT", bufs=6))
        out_pool = ctx.enter_context(tc.tile_pool(name="out", bufs=2))
        qkv_ps = ctx.enter_context(tc.tile_pool(name="qkv_ps", bufs=2, space="PSUM"))
        sc_ps = ctx.enter_context(tc.tile_pool(name="sc_ps", bufs=2, space="PSUM"))
        cx_ps = ctx.enter_context(tc.tile_pool(name="cx_ps", bufs=2, space="PSUM"))

        # ---- persistent SBUF tensors ----
        xT_c = [persist.tile([P, S], bf16, name=f"xT{c}") for c in range(NCH)]
        wq_c = [persist.tile([P, NCH, P], bf16, name=f"wq{c}") for c in range(NCH)]
        wk_c = [persist.tile([P, NCH, P], bf16, name=f"wk{c}") for c in range(NCH)]
        wv_c = [persist.tile([P, HID], bf16, name=f"wv{c}") for c in range(NCH)]
        v_sb = persist.tile([P, NKT, VROW], bf16)  # [p(seq), seq_chunk, 16*(64+1)]
        bq_sb = persist.tile([P, NCH], fp32)
        bk_sb = persist.tile([P, NCH], fp32)
        mask_sb = persist.tile([P, NKT], fp32)
        bv_sb = persist.tile([P, HID], fp32, name="bv_sb") if has_bv else None

        # ---- input DMAs, latency-ordered ----
        nc.sync.dma_start(out=bq_sb, in_=bq[:, :])
        nc.sync.dma_start(out=bk_sb, in_=bk[:, :])
        nc.sync.dma_start(out=mask_sb, in_=mask[:, :])
        if has_bv:
            bv_bcast = bass.AP(tensor=bv.tensor if hasattr(bv, "tensor") else bv,
                               offset=0, ap=[[0, P], [1, HID]])
            nc.sync.dma_start(out=bv_sb, in_=bv_bcast)
        for c in range(NCH):
            nc.sync.dma_start(out=xT_c[c], in_=xT[c * P:(c + 1) * P, :])
        nc.sync.dma_start(out=wq_c[0], in_=wq[0])
        nc.sync.dma_start(out=wk_c[0], in_=wk[0])
        for c in range(NCH):
            nc.sync.dma_start(out=wv_c[c], in_=wv[c * P:(c + 1) * P, :])
        for c in range(1, NCH):
            nc.sync.dma_start(out=wq_c[c], in_=wq[c])
            nc.sync.dma_start(out=wk_c[c], in_=wk[c])

        # ones columns for the softmax denominator live at col 64 of each
        # 65-wide head block; V copies below only overwrite cols 0..63
        nc.gpsimd.memset(v_sb, 1.0)

        # warmup matmuls on scratch data while the input DMAs stream in:
        # keeps the PE busy (and its clock ramping toward max) until xT +
        # the first weight column land, so Q(0)/K(0) run at full speed
        wscr = persist.tile([P, 512], bf16, name="warm_scratch")
        nc.vector.memset(wscr, 0.5)
        for _ in range(24):
            wps = sc_ps.tile([P, S], fp32, name="score_psum")
            nc.tensor.matmul(
                wps[:, 0:512],
                lhsT=wscr[:, 0:P],
                rhs=wscr,
                start=True,
                stop=True,
            )

        qT_tiles = {}
        kT_tiles = {}

        def qk_quarter(c, w_c, b_sb, dst_tiles, half, quarter):
            # one quarter (4 contraction chunks) of a Q/K projection half;
            # quarter 1 finishes the group and drains it (+bias) to SBUF
            if half == 0 and quarter == 0 and c not in dst_tiles:
                pool = qT_pool if dst_tiles is qT_tiles else kT_pool
                dst_tiles[c] = pool.tile([P, S], bf16, name="qkT")
            key = (id(dst_tiles), c, half)
            if quarter == 0:
                qk_quarter.ps[key] = qkv_ps.tile([P, 512], fp32, name="qkv_psum")
            ps = qk_quarter.ps[key]
            for kc in range(4 * quarter, 4 * quarter + 4):
                nc.tensor.matmul(
                    ps,
                    lhsT=w_c[c][:, kc, :],
                    rhs=xT_c[kc][:, half * 512:(half + 1) * 512],
                    start=(kc == 0),
                    stop=(kc == NCH - 1),
                )
            if quarter == 1:
                nc.vector.tensor_scalar_add(
                    out=dst_tiles[c][:, half * 512:(half + 1) * 512],
                    in0=ps,
                    scalar1=b_sb[:, c:c + 1],
                )
                del qk_quarter.ps[key]
        qk_quarter.ps = {}

        def v_half(st, half):
            # v_sb[:, st, heads half] = (X @ Wv)[:, half] (+bv)
            ps = qkv_ps.tile([P, 512], fp32, name="qkv_psum")
            for kc in range(NCH):
                nc.tensor.matmul(
                    ps,
                    lhsT=xT_c[kc][:, st * P:(st + 1) * P],
                    rhs=wv_c[kc][:, half * 512:(half + 1) * 512],
                    start=(kc == 0),
                    stop=(kc == NCH - 1),
                )
            dst = (
                v_sb[:, st, :]
                .rearrange("p (h x) -> p h x", x=HD + 1)[:, half * 8:(half + 1) * 8, 0:HD]
            )
            src = ps.rearrange("p (h x) -> p h x", x=HD)
            if has_bv:
                bvs = (
                    bv_sb[:, half * 512:(half + 1) * 512]
                    .rearrange("p (h x) -> p h x", x=HD)
                )
                nc.vector.tensor_add(out=dst, in0=src, in1=bvs)
            else:
                nc.vector.tensor_copy(out=dst, in_=src)

        def score_exp_kt(c, kt, pT_pair):
            # scores + exp for both heads of chunk c at key tile kt.
            # Per (head, q-half): two col-tiled 64x64-stationary matmuls
            # (keys 0:64 -> PSUM partitions 0:64 at tile col 0; keys 64:128
            # -> partitions 64:128 at tile col 64) sharing the moving Q
            # stream -> they run concurrently on the PE.
            qT_t, kT_t = qT_tiles[c], kT_tiles[c]
            for sub in range(2):
                po = 64 * sub
                ps = sc_ps.tile([P, S], fp32, name="score_psum")
                for half in range(2):
                    for kg in range(2):
                        nc.tensor.matmul(
                            ps[kg * 64:(kg + 1) * 64, half * 512:(half + 1) * 512],
                            lhsT=kT_t[po:po + 64, kt * P + kg * 64:kt * P + (kg + 1) * 64],
                            rhs=qT_t[po:po + 64, half * 512:(half + 1) * 512],
                            start=True,
                            stop=True,
                        )
                nc.scalar.activation(
                    out=pT_pair[sub][:, kt, :],
                    in_=ps,
                    func=EXP,
                    bias=mask_sb[:, kt:kt + 1],
                    scale=SCALE,
                )

        def ctx_quarter(h, pT_h, head_out, qt_base):
            # two qt context groups + normalization for head h
            for qt in (qt_base, qt_base + 1):
                cps = cx_ps.tile([P, HD + 1], fp32, name="ctx_psum")
                for kc in range(NKT):
                    nc.tensor.matmul(
                        cps,
                        lhsT=pT_h[:, kc, qt * P:(qt + 1) * P],
                        rhs=v_sb[:, kc, h * (HD + 1):(h + 1) * (HD + 1)],
                        start=(kc == 0),
                        stop=(kc == NKT - 1),
                    )
                recip = misc.tile([P, 1], fp32, name="recip")
                nc.vector.reciprocal(recip, cps[:, HD:HD + 1])
                nc.vector.tensor_scalar_mul(
                    out=head_out[:, qt, :],
                    in0=cps[:, 0:HD],
                    scalar1=recip,
                )
                nc.sync.dma_start(
                    out=out[qt * P:(qt + 1) * P, h * HD:(h + 1) * HD],
                    in_=head_out[:, qt, :],
                )

        # ---- pipeline ----
        # per-chunk filler slots, one per kt iteration: QK(c+1) quarters,
        # V halves (chunks 0-2), ctx(c-2) quarters (chunks 2-7 + tail)
        v_jobs = [(st, half) for st in range(NKT) for half in range(2)]
        v_sched = {0: v_jobs[0:5], 1: v_jobs[5:11], 2: v_jobs[11:16]}
        pT_live = {}

        def chunk_fillers(c):
            jobs = []
            if c + 1 < NCH:
                for half in range(2):
                    for quarter in range(2):
                        jobs.append(("qk", (c + 1, wq_c, bq_sb, qT_tiles, half, quarter)))
                for half in range(2):
                    for quarter in range(2):
                        jobs.append(("qk", (c + 1, wk_c, bk_sb, kT_tiles, half, quarter)))
            for st, half in v_sched.get(c, []):
                jobs.append(("v", (st, half)))
            ctx_chunks = []
            if 2 <= c <= 6:
                ctx_chunks.append(c - 2)
            if c == 7:
                ctx_chunks.extend((5, 6))
            for cc in ctx_chunks:
                pA, pB = pT_live.pop(cc)
                oA = out_pool.tile([P, NQT, HD], fp32, name="head_out")
                oB = out_pool.tile([P, NQT, HD], fp32, name="head_out")
                for qt_base in range(0, NQT, 2):
                    jobs.append(("ctx", (2 * cc, pA, oA, qt_base)))
                    jobs.append(("ctx", (2 * cc + 1, pB, oB, qt_base)))
            return jobs

        def run_job(job):
            kind, args = job
            if kind == "qk":
                qk_quarter(*args)
            elif kind == "v":
                v_half(*args)
            else:
                ctx_quarter(*args)

        # Q(0)/K(0) ahead of the stream
        for half in range(2):
            for quarter in range(2):
                qk_quarter(0, wq_c, bq_sb, qT_tiles, half, quarter)
        for half in range(2):
            for quarter in range(2):
                qk_quarter(0, wk_c, bk_sb, kT_tiles, half, quarter)

        for c in range(NCH):
            pT_pair = (
                pT_pool.tile([P, NKT, S], bf16, name="pT"),
                pT_pool.tile([P, NKT, S], bf16, name="pT"),
            )
            pT_live[c] = pT_pair
            jobs = chunk_fillers(c)
            # spread filler jobs evenly across the 8 kt iterations
            per_kt = [[] for _ in range(NKT)]
            for i, job in enumerate(jobs):
                per_kt[i * NKT // len(jobs)].append(job)
            for kt in range(NKT):
                score_exp_kt(c, kt, pT_pair)
                for job in per_kt[kt]:
                    run_job(job)
            qT_tiles.pop(c)
            kT_tiles.pop(c)

        # tail: last head pair
        pA, pB = pT_live.pop(7)
        oA = out_pool.tile([P, NQT, HD], fp32, name="head_out")
        oB = out_pool.tile([P, NQT, HD], fp32, name="head_out")
        for qt_base in range(0, NQT, 2):
            ctx_quarter(14, pA, oA, qt_base)
            ctx_quarter(15, pB, oB, qt_base)

    nc.finalize()
    return nc


def _prep_inputs(inputs):
    bf16 = ml_dtypes.bfloat16
    hs = np.asarray(inputs["hidden_states"], dtype=np.float32)
    am = np.asarray(inputs["attention_mask"], dtype=np.float32)
    Wq = np.asarray(inputs["Wq"], dtype=np.float32)
    Wk = np.asarray(inputs["Wk"], dtype=np.float32)
    Wv = np.asarray(inputs["Wv"], dtype=np.float32)
    bq = np.asarray(inputs["bq"], dtype=np.float32)
    bk = np.asarray(inputs["bk"], dtype=np.float32)
    bv = np.asarray(inputs["bv"], dtype=np.float32)

    has_bv = bool(np.any(bv))

    # [hid_in, hid_out] -> [c_out, p(hid_in%128), kc(hid_in//128), col]
    def col_shuffle(w):
        return np.ascontiguousarray(
            w.astype(bf16).reshape(NCH, P, NCH, P).transpose(2, 1, 0, 3)
        )

    wq_b = col_shuffle(Wq)
    wk_b = col_shuffle(Wk)
    wv_b = np.ascontiguousarray(Wv.astype(bf16))
    bq_c = np.ascontiguousarray(bq.reshape(NCH, P).T)
    bk_c = np.ascontiguousarray(bk.reshape(NCH, P).T)

    hs_b = hs.astype(bf16)
    in_maps = []
    for b in range(B):
        m = {
            "xT": np.ascontiguousarray(hs_b[b].T),
            "wq": wq_b,
            "wk": wk_b,
            "wv": wv_b,
            "bq": bq_c,
            "bk": bk_c,
            "mask": np.ascontiguousarray(am[b, 0, 0].reshape(NKT, P).T),
        }
        if has_bv:
            m["bv"] = bv
        in_maps.append(m)
    return in_maps, has_bv


def _run(inputs, trace=False, trace_cores=None):
    from concourse.bass_utils import run_bass_kernel_spmd

    in_maps, has_bv = _prep_inputs(inputs)
    nc = _build(has_bv)
    res = run_bass_kernel_spmd(
        nc, in_maps, core_ids=list(range(N_CORES)), trace=trace,
        trace_cores=trace_cores,
    )
    out = np.stack([np.asarray(r["out"], dtype=np.float32) for r in res.results])
    return out, res


def kernel(**inputs) -> np.ndarray:
    out, _ = _run(inputs, trace=False)
    return out


# revision 9
# speedup vs baseline: 1.0445x; 1.0445x over previous
"""BERT self-attention on 8 Trainium2 NeuronCores.

Sharding: data-parallel over batch (B=8 -> 1 batch element per core).
Every core runs the same single-core Bass kernel on its own batch slice;
weights/mask are replicated. The final output is a host-side stack.

Per-core algorithm (S=1024, HID=1024, NH=16, HD=64), all matmuls bf16
with fp32 PSUM accumulation:

  xT = X^T (host-transposed, bf16)             [HID, S]
  Q^T = Wq^T @ X^T   (lhsT = Wq col-chunks)    [HID, S]  (+bq per-partition)
  K^T = Wk^T @ X^T                             [HID, S]  (+bk per-partition)
  V   = X @ Wv       (lhsT = xT)               [S, HID]  (+bv broadcast)
  per head pair (2c, 2c+1) living in hid chunk c (head 2c in partitions
  0:64, head 2c+1 in 64:128 of qT/kT chunk c):
    S^T = K_h @ Q_h^T  -- per (head, kt, q-half): TWO col-tiled matmuls
          with 64-row contraction and 64-col stationaries at PE
          tile_position (r, 0) and (r, 64); they share the moving Q
          stream and run CONCURRENTLY on the PE array, so a head's
          512-q-col score block costs ~226ns instead of ~452ns.
    P^T = exp(S^T/8 + mask[k])   (ScalarE; the bottleneck: 128 exps x
          ~1.1us = 142us of ACT time; the schedule is built to start
          this stream early and keep it dense)
    ctx = P^T.T @ [V_h | 1]  (ones column yields the softmax denominator)
    out[:, h] = ctx[:, :64] * (1/Z)

Pipeline: wq/wk are DMA'd as column chunks so Q(0)/K(0) start right
after xT lands (~14us); warmup matmuls keep the PE clock ramped through
the fill; V projections and QK(c+1) are emitted in small per-kt slots
inside each chunk's score/exp loop so no blob ever blocks the in-order
PE queue ahead of the exp stream; ctx for chunk c runs under chunk
c+2's exp window (c7 carries ctx(5) and ctx(6); only ctx(7) trails).
"""

import functools

import numpy as np
import ml_dtypes

B, S, HID = 8, 1024, 1024
NH, HD = 16, 64
P = 128
NCH = HID // P  # hid chunks (8)
NKT = S // P  # key tiles (8)
NQT = S // P  # query tiles (8)
VROW = NH * (HD + 1)  # 1040: per-seq-chunk V row: 16 x (64 V cols + ones col)
N_CORES = 8

SCALE = 1.0 / float(np.sqrt(HD))


@functools.lru_cache(maxsize=None)
def _build(has_bv: bool):
    import concourse.bass as bass
    import concourse.tile as tile
    from concourse import bacc, mybir
    from contextlib import ExitStack

    fp32 = mybir.dt.float32
    bf16 = mybir.dt.bfloat16
    EXP = mybir.ActivationFunctionType.Exp

    nc = bacc.Bacc("TRN2", target_bir_lowering=False)

    xT = nc.dram_tensor("xT", [HID, S], bf16, kind="ExternalInput")
    # wq/wk host-pre-shuffled to [c_out, p(hid_in%128), kc(hid_in//128), col]
    # so one contiguous DMA delivers the full column chunk c (everything
    # Q(c)/K(c) need), letting chunk 0's projections start right after xT.
    wq = nc.dram_tensor("wq", [NCH, P, NCH, P], bf16, kind="ExternalInput")
    wk = nc.dram_tensor("wk", [NCH, P, NCH, P], bf16, kind="ExternalInput")
    wv = nc.dram_tensor("wv", [HID, HID], bf16, kind="ExternalInput")
    bq = nc.dram_tensor("bq", [P, NCH], fp32, kind="ExternalInput")
    bk = nc.dram_tensor("bk", [P, NCH], fp32, kind="ExternalInput")
    bv = nc.dram_tensor("bv", [HID], fp32, kind="ExternalInput") if has_bv else None
    mask = nc.dram_tensor("mask", [P, NKT], fp32, kind="ExternalInput")
    out = nc.dram_tensor("out", [S, HID], fp32, kind="ExternalOutput")

    with tile.TileContext(nc) as tc, ExitStack() as ctx:
        persist = ctx.enter_context(tc.tile_pool(name="persist", bufs=1))
        misc = ctx.enter_context(tc.tile_pool(name="misc", bufs=8))
        qT_pool = ctx.enter_context(tc.tile_pool(name="qT", bufs=2))
        kT_pool = ctx.enter_context(tc.tile_pool(name="kT", bufs=2))
        pT_pool = ctx.enter_context(tc.tile_pool(name="pT", bufs=6))
        # bufs=4: chunk 7 has two ctx chunks (4 head_out tiles) in flight;
        # with fewer bufs a scale write would wait on an out-DMA emitted
        # behind it in the same engine queue (deadlock, caught by CoreSim)
        out_pool = ctx.enter_context(tc.tile_pool(name="out", bufs=4))
        qkv_ps = ctx.enter_context(tc.tile_pool(name="qkv_ps", bufs=2, space="PSUM"))
        sc_ps = ctx.enter_context(tc.tile_pool(name="sc_ps", bufs=2, space="PSUM"))
        cx_ps = ctx.enter_context(tc.tile_pool(name="cx_ps", bufs=2, space="PSUM"))

        # ---- persistent SBUF tensors ----
        xT_c = [persist.tile([P, S], bf16, name=f"xT{c}") for c in range(NCH)]
        wq_c = [persist.tile([P, NCH, P], bf16, name=f"wq{c}") for c in range(NCH)]
        wk_c = [persist.tile([P, NCH, P], bf16, name=f"wk{c}") for c in range(NCH)]
        wv_c = [persist.tile([P, HID], bf16, name=f"wv{c}") for c in range(NCH)]
        v_sb = persist.tile([P, NKT, VROW], bf16)  # [p(seq), seq_chunk, 16*(64+1)]
        bq_sb = persist.tile([P, NCH], fp32)
        bk_sb = persist.tile([P, NCH], fp32)
        mask_sb = persist.tile([P, NKT], fp32)
        bv_sb = persist.tile([P, HID], fp32, name="bv_sb") if has_bv else None

        # ---- input DMAs, latency-ordered ----
        nc.sync.dma_start(out=bq_sb, in_=bq[:, :])
        nc.sync.dma_start(out=bk_sb, in_=bk[:, :])
        nc.sync.dma_start(out=mask_sb, in_=mask[:, :])
        if has_bv:
            bv_bcast = bass.AP(tensor=bv.tensor if hasattr(bv, "tensor") else bv,
                               offset=0, ap=[[0, P], [1, HID]])
            nc.sync.dma_start(out=bv_sb, in_=bv_bcast)
        for c in range(NCH):
            nc.sync.dma_start(out=xT_c[c], in_=xT[c * P:(c + 1) * P, :])
        nc.sync.dma_start(out=wq_c[0], in_=wq[0])
        nc.sync.dma_start(out=wk_c[0], in_=wk[0])
        for c in range(NCH):
            nc.sync.dma_start(out=wv_c[c], in_=wv[c * P:(c + 1) * P, :])
        for c in range(1, NCH):
            nc.sync.dma_start(out=wq_c[c], in_=wq[c])
            nc.sync.dma_start(out=wk_c[c], in_=wk[c])

        # ones columns for the softmax denominator live at col 64 of each
        # 65-wide head block; V copies below only overwrite cols 0..63
        nc.gpsimd.memset(v_sb, 1.0)

        # warmup matmuls on scratch data while the input DMAs stream in:
        # keeps the PE busy (and its clock ramping toward max) until xT +
        # the first weight column land, so Q(0)/K(0) run at full speed
        wscr = persist.tile([P, 512], bf16, name="warm_scratch")
        nc.vector.memset(wscr, 0.5)
        for _ in range(12):
            wps = sc_ps.tile([P, S], fp32, name="score_psum")
            nc.tensor.matmul(
                wps[:, 0:512],
                lhsT=wscr[:, 0:P],
                rhs=wscr,
                start=True,
                stop=True,
            )

        qT_tiles = {}
        kT_tiles = {}

        def qk_half(c, w_c, b_sb, dst_tiles, half):
            # one q-half of a Q/K projection: 8-matmul PSUM group + bias
            # drain to SBUF (whole group in one filler slot so the qkv
            # pool rotation never interleaves two open groups)
            if c not in dst_tiles:
                pool = qT_pool if dst_tiles is qT_tiles else kT_pool
                dst_tiles[c] = pool.tile([P, S], bf16, name="qkT")
            ps = qkv_ps.tile([P, 512], fp32, name="qkv_psum")
            for kc in range(NCH):
                nc.tensor.matmul(
                    ps,
                    lhsT=w_c[c][:, kc, :],
                    rhs=xT_c[kc][:, half * 512:(half + 1) * 512],
                    start=(kc == 0),
                    stop=(kc == NCH - 1),
                )
            nc.vector.tensor_scalar_add(
                out=dst_tiles[c][:, half * 512:(half + 1) * 512],
                in0=ps,
                scalar1=b_sb[:, c:c + 1],
            )

        def v_half(st, half):
            # v_sb[:, st, heads half] = (X @ Wv)[:, half] (+bv)
            ps = qkv_ps.tile([P, 512], fp32, name="qkv_psum")
            for kc in range(NCH):
                nc.tensor.matmul(
                    ps,
                    lhsT=xT_c[kc][:, st * P:(st + 1) * P],
                    rhs=wv_c[kc][:, half * 512:(half + 1) * 512],
                    start=(kc == 0),
                    stop=(kc == NCH - 1),
                )
            dst = (
                v_sb[:, st, :]
                .rearrange("p (h x) -> p h x", x=HD + 1)[:, half * 8:(half + 1) * 8, 0:HD]
            )
            src = ps.rearrange("p (h x) -> p h x", x=HD)
            if has_bv:
                bvs = (
                    bv_sb[:, half * 512:(half + 1) * 512]
                    .rearrange("p (h x) -> p h x", x=HD)
                )
                nc.vector.tensor_add(out=dst, in0=src, in1=bvs)
            else:
                nc.vector.tensor_copy(out=dst, in_=src)

        def score_exp_kt(c, kt, pT_pair):
            # scores + exp for both heads of chunk c at key tile kt.
            # Per (head, q-half): two col-tiled 64x64-stationary matmuls
            # (keys 0:64 -> PSUM partitions 0:64 at tile col 0; keys 64:128
            # -> partitions 64:128 at tile col 64) sharing the moving Q
            # stream -> they run concurrently on the PE.
            qT_t, kT_t = qT_tiles[c], kT_tiles[c]
            for sub in range(2):
                po = 64 * sub
                ps = sc_ps.tile([P, S], fp32, name="score_psum")
                for half in range(2):
                    for kg in range(2):
                        nc.tensor.matmul(
                            ps[kg * 64:(kg + 1) * 64, half * 512:(half + 1) * 512],
                            lhsT=kT_t[po:po + 64, kt * P + kg * 64:kt * P + (kg + 1) * 64],
                            rhs=qT_t[po:po + 64, half * 512:(half + 1) * 512],
                            start=True,
                            stop=True,
                        )
                nc.scalar.activation(
                    out=pT_pair[sub][:, kt, :],
                    in_=ps,
                    func=EXP,
                    bias=mask_sb[:, kt:kt + 1],
                    scale=SCALE,
                )

        def ctx_quarter(h, pT_h, head_out, qt_base):
            # two qt context groups + normalization for head h
            for qt in (qt_base, qt_base + 1):
                cps = cx_ps.tile([P, HD + 1], fp32, name="ctx_psum")
                for kc in range(NKT):
                    nc.tensor.matmul(
                        cps,
                        lhsT=pT_h[:, kc, qt * P:(qt + 1) * P],
                        rhs=v_sb[:, kc, h * (HD + 1):(h + 1) * (HD + 1)],
                        start=(kc == 0),
                        stop=(kc == NKT - 1),
                    )
                recip = misc.tile([P, 1], fp32, name="recip")
                nc.vector.reciprocal(recip, cps[:, HD:HD + 1])
                nc.vector.tensor_scalar_mul(
                    out=head_out[:, qt, :],
                    in0=cps[:, 0:HD],
                    scalar1=recip,
                )
                nc.sync.dma_start(
                    out=out[qt * P:(qt + 1) * P, h * HD:(h + 1) * HD],
                    in_=head_out[:, qt, :],
                )

        # ---- pipeline ----
        # per-chunk filler jobs, spread over the 8 kt iterations: QK(c+1)
        # halves, V halves (chunks 0-1), ctx(c-2) quarters (chunk 7 carries
        # ctx(5) and ctx(6); ctx(7) trails the stream)
        v_jobs = [(st, half) for st in range(NKT) for half in range(2)]
        pT_live = {}

        def chunk_fillers(c):
            jobs = []
            if c + 1 < NCH:
                for w_c, b_sb, dst in ((wq_c, bq_sb, qT_tiles), (wk_c, bk_sb, kT_tiles)):
                    for half in range(2):
                        jobs.append(("qk", (c + 1, w_c, b_sb, dst, half)))
            if c == 0:
                jobs.extend(("v", vj) for vj in v_jobs[0:8])
            elif c == 1:
                jobs.extend(("v", vj) for vj in v_jobs[8:16])
            ctx_chunks = []
            if 2 <= c <= 6:
                ctx_chunks.append(c - 2)
            if c == 7:
                ctx_chunks.extend((5, 6))
            for cc in ctx_chunks:
                pA, pB = pT_live.pop(cc)
                oA = out_pool.tile([P, NQT, HD], fp32, name="head_out")
                oB = out_pool.tile([P, NQT, HD], fp32, name="head_out")
                for qt_base in range(0, NQT, 2):
                    jobs.append(("ctx", (2 * cc, pA, oA, qt_base)))
                    jobs.append(("ctx", (2 * cc + 1, pB, oB, qt_base)))
            return jobs

        def run_job(job):
            kind, args = job
            if kind == "qk":
                qk_half(*args)
            elif kind == "v":
                v_half(*args)
            else:
                ctx_quarter(*args)

        # Q(0)/K(0) ahead of the stream
        for half in range(2):
            qk_half(0, wq_c, bq_sb, qT_tiles, half)
        for half in range(2):
            qk_half(0, wk_c, bk_sb, kT_tiles, half)

        for c in range(NCH):
            pT_pair = (
                pT_pool.tile([P, NKT, S], bf16, name="pT"),
                pT_pool.tile([P, NKT, S], bf16, name="pT"),
            )
            pT_live[c] = pT_pair
            jobs = chunk_fillers(c)
            # round-robin: at most ceil(n/8) filler jobs between kt steps
            per_kt = [[] for _ in range(NKT)]
            for i, job in enumerate(jobs):
                per_kt[i % NKT].append(job)
            for kt in range(NKT):
                score_exp_kt(c, kt, pT_pair)
                for job in per_kt[kt]:
                    run_job(job)
            qT_tiles.pop(c)
            kT_tiles.pop(c)

        # tail: last head pair
        pA, pB = pT_live.pop(7)
        oA = out_pool.tile([P, NQT, HD], fp32, name="head_out")
        oB = out_pool.tile([P, NQT, HD], fp32, name="head_out")
        for qt_base in range(0, NQT, 2):
            ctx_quarter(14, pA, oA, qt_base)
            ctx_quarter(15, pB, oB, qt_base)

    nc.finalize()
    return nc


def _prep_inputs(inputs):
    bf16 = ml_dtypes.bfloat16
    hs = np.asarray(inputs["hidden_states"], dtype=np.float32)
    am = np.asarray(inputs["attention_mask"], dtype=np.float32)
    Wq = np.asarray(inputs["Wq"], dtype=np.float32)
    Wk = np.asarray(inputs["Wk"], dtype=np.float32)
    Wv = np.asarray(inputs["Wv"], dtype=np.float32)
    bq = np.asarray(inputs["bq"], dtype=np.float32)
    bk = np.asarray(inputs["bk"], dtype=np.float32)
    bv = np.asarray(inputs["bv"], dtype=np.float32)

    has_bv = bool(np.any(bv))

    # [hid_in, hid_out] -> [c_out, p(hid_in%128), kc(hid_in//128), col]
    def col_shuffle(w):
        return np.ascontiguousarray(
            w.astype(bf16).reshape(NCH, P, NCH, P).transpose(2, 1, 0, 3)
        )

    wq_b = col_shuffle(Wq)
    wk_b = col_shuffle(Wk)
    wv_b = np.ascontiguousarray(Wv.astype(bf16))
    bq_c = np.ascontiguousarray(bq.reshape(NCH, P).T)
    bk_c = np.ascontiguousarray(bk.reshape(NCH, P).T)

    hs_b = hs.astype(bf16)
    in_maps = []
    for b in range(B):
        m = {
            "xT": np.ascontiguousarray(hs_b[b].T),
            "wq": wq_b,
            "wk": wk_b,
            "wv": wv_b,
            "bq": bq_c,
            "bk": bk_c,
            "mask": np.ascontiguousarray(am[b, 0, 0].reshape(NKT, P).T),
        }
        if has_bv:
            m["bv"] = bv
        in_maps.append(m)
    return in_maps, has_bv


def _run(inputs, trace=False, trace_cores=None):
    from concourse.bass_utils import run_bass_kernel_spmd

    in_maps, has_bv = _prep_inputs(inputs)
    nc = _build(has_bv)
    res = run_bass_kernel_spmd(
        nc, in_maps, core_ids=list(range(N_CORES)), trace=trace,
        trace_cores=trace_cores,
    )
    out = np.stack([np.asarray(r["out"], dtype=np.float32) for r in res.results])
    return out, res


def kernel(**inputs) -> np.ndarray:
    out, _ = _run(inputs, trace=False)
    return out
